# revision 1
# baseline (speedup 1.0000x reference)
"""Trainium2 Bass kernel for nn_MemoryLayer (embedding_lookup).

Reference computation (per token t, chunk k of 64):
  h[t,k]  = sum_i (x[t, k*16+i] >= 0) * 2^(15-i)          (16-bit hash)
  p[t,k]  = prod_i sigmoid(2 * x[t, k*16+i])               (gate)
  out[t, k*32:(k+1)*32] = tables[k, h[t,k], :] * p[t,k]

Sharding: expert-parallel over 8 cores. Core c owns chunks [8c, 8c+8):
its x slice [8192, 128], its 8 tables, and output columns [256c, 256c+256).

Per-core kernel:
  - hash/gate on DVE/ACT (features along free dim, 128 tokens/partition)
  - gather via dma_gather ucode: tables viewed as [32768, 64] pair-rows
    (256 B elems), idx = h>>1 as int16, one gather of 8192 idxs per chunk
  - idx arrays need the ucode's [n%16, n//16] 16-partition wrapped layout,
    replicated x8 down partitions: built with 8 PE selector matmuls
    (partition fold 128->16) + a replication matmul (16->128)
  - parity select + gate fused: out = even*(p*(1-par)) + odd*(p*par)
"""
import sys

sys.path.insert(0, "/opt/trn_rl_repo")

import numpy as np

import concourse.bacc as bacc
import concourse.bass as bass
import concourse.mybir as mybir
import concourse.tile as tile
from concourse import bass_utils
from concourse.library_config import mlp

P = 128
KLOC = 8  # chunks per core
V = 65536  # buckets per table
V2 = V // 2  # pair rows
E = 64  # f32 per pair row (256 B)
OC = 32  # out chunk
F32 = mybir.dt.float32
I16 = mybir.dt.int16
ALU = mybir.AluOpType
ACT = mybir.ActivationFunctionType


def build_program(ntok=8192, repeats=1, skip=(), gn=1024, gsp=True, gq=4, scratch=16384):
    """Build the per-core SPMD Bass program. ntok must be a multiple of 256.

    skip: subset of {"hash","gate","idx","gather","select","store"} for
    ablation timing (skipped stages leave garbage downstream; timing only).
    """
    jt = ntok // P  # total j blocks
    jh = jt // 2  # j blocks per half
    nc = bacc.Bacc("TRN2", target_bir_lowering=False, debug=False,
                   num_swdge_queues=gq, dynamic_dma_scratch_size=scratch)

    x_d = nc.dram_tensor("x", [ntok, P], F32, kind="ExternalInput")
    tab_d = nc.dram_tensor("tab", [KLOC * V2, E], F32, kind="ExternalInput")
    w_d = nc.dram_tensor("w", [P, P], F32, kind="ExternalInput")
    eye_d = nc.dram_tensor("eye", [P, P], F32, kind="ExternalInput")
    r16_d = nc.dram_tensor("r16", [16, P], F32, kind="ExternalInput")
    out_d = nc.dram_tensor("out", [ntok, KLOC * OC], F32, kind="ExternalOutput")
    idx_dram = (
        nc.dram_tensor("idxin", [P, KLOC * (ntok // 16)], I16, kind="ExternalInput")
        if "idxdram" in skip
        else None
    )

    with tile.TileContext(nc) as tc:
        nc.gpsimd.load_library(mlp)
        with (
            tc.tile_pool(name="const", bufs=1) as cp,
            tc.tile_pool(name="xp", bufs=2) as xp,
            tc.tile_pool(name="wsg", bufs=1) as wsgp,
            tc.tile_pool(name="hp", bufs=2) as hpp,
            tc.tile_pool(name="small", bufs=2) as sp,
            tc.tile_pool(name="hrs", bufs=2) as hrsp,
            tc.tile_pool(name="gt", bufs=3) as gp,
            tc.tile_pool(name="tmp", bufs=2) as tp,
            tc.tile_pool(name="big", bufs=2) as bp,
            tc.tile_pool(name="psA", bufs=1, space="PSUM") as psA,
            tc.tile_pool(name="psB", bufs=1, space="PSUM") as psB,
        ):
            w_t = cp.tile([P, P], F32)
            nc.sync.dma_start(out=w_t[:], in_=w_d[:])
            eye_t = cp.tile([P, P], F32)
            nc.sync.dma_start(out=eye_t[:], in_=eye_d[:])
            r16_t = cp.tile([16, P], F32)
            nc.sync.dma_start(out=r16_t[:], in_=r16_d[:])

            def pair_tree_mult(out_ap, src, jhn):
                """out = prod over i of src[p, j, (k i)] (i = 16), pairwise."""
                sg5 = src.rearrange("p j (k i two) -> p j k i two", k=KLOC, two=2)
                t1 = hpp.tile([P, jhn, KLOC, 8], F32, tag="t1")
                nc.vector.tensor_tensor(
                    out=t1[:],
                    in0=sg5[:, :, :, :, 0:1].rearrange("p j k i o -> p j k (i o)"),
                    in1=sg5[:, :, :, :, 1:2].rearrange("p j k i o -> p j k (i o)"),
                    op=ALU.mult,
                )
                t15 = t1[:].rearrange("p j k (i two) -> p j k i two", i=4, two=2)
                t2 = hpp.tile([P, jhn, KLOC, 4], F32, tag="t2")
                nc.vector.tensor_tensor(
                    out=t2[:],
                    in0=t15[:, :, :, :, 0:1].rearrange("p j k i o -> p j k (i o)"),
                    in1=t15[:, :, :, :, 1:2].rearrange("p j k i o -> p j k (i o)"),
                    op=ALU.mult,
                )
                t25 = t2[:].rearrange("p j k (i two) -> p j k i two", i=2, two=2)
                t3 = hpp.tile([P, jhn, KLOC, 2], F32, tag="t3")
                nc.vector.tensor_tensor(
                    out=t3[:],
                    in0=t25[:, :, :, :, 0:1].rearrange("p j k i o -> p j k (i o)"),
                    in1=t25[:, :, :, :, 1:2].rearrange("p j k i o -> p j k (i o)"),
                    op=ALU.mult,
                )
                nc.vector.tensor_tensor(
                    out=out_ap,
                    in0=t3[:, :, :, 0:1],
                    in1=t3[:, :, :, 1:2],
                    op=ALU.mult,
                )

            def front_end(h):
                """x load + hash + gate + idx prep for half h. Returns
                (idx16_h, pe_h, po_h) tiles (None entries when skipped)."""
                jb = h * jh
                x_t = xp.tile([P, jh, P], F32, tag="x")
                nc.sync.dma_start(
                    out=x_t[:],
                    in_=x_d[:].rearrange("(p j) f -> p j f", j=jt)[
                        :, jb:jb + jh, :
                    ],
                )
                x4 = x_t[:].rearrange("p j (k i) -> p j k i", i=16)

                idx16_h = pe_h = po_h = None
                if "hash" not in skip:
                    # wb = (x >= 0) * W ; hp = segsum(wb)  (= h>>1)
                    wb = wsgp.tile([P, jh, P], F32, tag="wsg")
                    nc.vector.scalar_tensor_tensor(
                        out=wb[:],
                        in0=x_t[:],
                        scalar=0.0,
                        in1=w_t[:]
                        .rearrange("p (o f) -> p o f", o=1)
                        .to_broadcast([P, jh, P]),
                        op0=ALU.is_ge,
                        op1=ALU.mult,
                    )
                    hp_t = hpp.tile([P, jh, KLOC], F32, tag="hp")
                    nc.vector.tensor_reduce(
                        out=hp_t[:],
                        in_=wb[:].rearrange("p j (k i) -> p j k i", i=16),
                        axis=mybir.AxisListType.X,
                        op=ALU.add,
                    )

                if "gate" not in skip:
                    # sg = sigmoid(2x); pt = segprod(sg); parity; pe/po
                    pt_t = sp.tile([P, KLOC, jh], F32, tag="pt")
                    pb_t = sp.tile([P, KLOC, jh], F32, tag="pb")
                    po_h = sp.tile([P, KLOC, jh], F32, tag="po")
                    pe_h = sp.tile([P, KLOC, jh], F32, tag="pe")
                    sg = wsgp.tile([P, jh, P], F32, tag="wsg")
                    nc.scalar.activation(sg[:], x_t[:], ACT.Sigmoid, scale=2.0)
                    pair_tree_mult(
                        pt_t[:].rearrange("p (k o) j -> p j k o", o=1),
                        sg[:],
                        jh,
                    )
                    nc.vector.tensor_scalar(
                        out=pb_t[:].rearrange("p (k o) j -> p j k o", o=1),
                        in0=x4[:, :, :, 15:16],
                        scalar1=0.0,
                        scalar2=None,
                        op0=ALU.is_ge,
                    )
                    nc.vector.tensor_tensor(
                        out=po_h[:], in0=pt_t[:], in1=pb_t[:], op=ALU.mult
                    )
                    nc.vector.tensor_tensor(
                        out=pe_h[:], in0=pt_t[:], in1=po_h[:], op=ALU.subtract
                    )

                if "idxdram" in skip:
                    idx16_h = bp.tile([P, KLOC, jh, 8], I16, tag="idx")
                    nc.sync.dma_start(
                        out=idx16_h[:],
                        in_=idx_dram[:].rearrange(
                            "p (k j g) -> p k j g", k=KLOC, j=jt, g=8
                        )[:, :, jb:jb + jh, :],
                    )
                elif "idx" not in skip and "hash" not in skip:
                    # [p=(g,q), (j,k)] -> wrapped [q, (k, j, g)] x8 replicas
                    idx16_h = bp.tile([P, KLOC, jh, 8], I16, tag="idx")
                    psT = psA.tile([16, 8, jh, KLOC], F32, tag="psT")
                    hp_flat = hp_t[:].rearrange("p j k -> p (j k)")
                    for g in range(8):
                        nc.tensor.matmul(
                            psT[:, g].rearrange("q j k -> q (j k)"),
                            lhsT=eye_t[:, g * 16:(g + 1) * 16],
                            rhs=hp_flat,
                            start=True,
                            stop=True,
                        )
                    hrs_t = hrsp.tile([16, KLOC, jh, 8], F32, tag="hrs")
                    nc.vector.tensor_copy(
                        out=hrs_t[:].rearrange("q k j g -> q g j k"), in_=psT[:]
                    )
                    ipx = psB.tile([P, KLOC * jh * 8], F32, tag="ipx")
                    hrs_flat = hrs_t[:].rearrange("q k j g -> q (k j g)")
                    tot = KLOC * jh * 8
                    nmm = max(tot // 512, 1)
                    mw = tot // nmm
                    for m in range(nmm):
                        nc.tensor.matmul(
                            ipx[:, m * mw:(m + 1) * mw],
                            lhsT=r16_t[:],
                            rhs=hrs_flat[:, m * mw:(m + 1) * mw],
                            start=True,
                            stop=True,
                        )
                    nc.vector.tensor_copy(
                        out=idx16_h[:],
                        in_=ipx[:].rearrange(
                            "p (k j g) -> p k j g", k=KLOC, j=jh, g=8
                        ),
                    )
                return idx16_h, pe_h, po_h

            def back_end(h, idx16_h, pe_h, po_h):
                """gathers + parity-select + gate + store for half h."""
                jb = h * jh
                res_h = bp.tile([P, jh, KLOC * OC], F32, tag="res")
                for k in range(KLOC):
                    if "gather" in skip and "select" in skip:
                        continue
                    gt_t = gp.tile([P, jh, E], F32, tag="gt")
                    if "gather" in skip:
                        nc.vector.memset(gt_t[:], 0.0)
                    else:
                        gne = min(gn, jh * P)
                        nsub = jh * P // gne
                        jn = gne // P
                        idx_flat = idx16_h[:, k].rearrange("p j g -> p (j g)")
                        for sub in range(nsub):
                            nc.gpsimd.dma_gather(
                                gt_t[:, sub * jn:(sub + 1) * jn, :],
                                tab_d[k * V2:(k + 1) * V2, :],
                                idx_flat[
                                    :, sub * (gne // 16):(sub + 1) * (gne // 16)
                                ],
                                gne,
                                gne,
                                E,
                                single_packet=gsp,
                                queue_num=(k * nsub + sub) % gq,
                            )
                    if "select" not in skip:
                        even = gt_t[:, :, 0:OC]
                        odd = gt_t[:, :, OC:E]
                        res_k = res_h[:, :, k * OC:(k + 1) * OC]
                        pe_b = (
                            pe_h[:, k, :]
                            .rearrange("p (j o) -> p j o", o=1)
                            .to_broadcast([P, jh, OC])
                        )
                        po_b = (
                            po_h[:, k, :]
                            .rearrange("p (j o) -> p j o", o=1)
                            .to_broadcast([P, jh, OC])
                        )
                        nc.vector.tensor_tensor(
                            out=res_k, in0=even, in1=pe_b, op=ALU.mult
                        )
                        tmp_t = tp.tile([P, jh, OC], F32, tag="tmp")
                        nc.vector.tensor_tensor(
                            out=tmp_t[:], in0=odd, in1=po_b, op=ALU.mult
                        )
                        nc.vector.tensor_tensor(
                            out=res_k, in0=res_k, in1=tmp_t[:], op=ALU.add
                        )

                if "store" not in skip and "select" not in skip:
                    nc.sync.dma_start(
                        out=out_d[:].rearrange("(p j) c -> p j c", j=jt)[
                            :, jb:jb + jh, :
                        ],
                        in_=res_h[:],
                    )

            def body():
                fe0 = front_end(0)
                back_end(0, *fe0)
                fe1 = front_end(1)
                back_end(1, *fe1)

            if repeats > 1:
                with tc.For_i(0, repeats, 1):
                    body()
            else:
                body()

    nc.compile()
    return nc


def make_consts():
    f = np.arange(P)
    i = f % 16
    w = np.where(i == 15, 0.0, 2.0 ** (14 - i)).astype(np.float32)
    w_full = np.tile(w[None, :], (P, 1))
    eye = np.eye(P, dtype=np.float32)
    r16 = (np.arange(P)[None, :] % 16 == np.arange(16)[:, None]).astype(np.float32)
    return w_full, eye, r16


def make_in_maps(x, tables):
    """x [B, S, 1024] f32, tables [64, 65536, 32] f32 -> 8 per-core dicts."""
    b, s, _ = x.shape
    xf = np.ascontiguousarray(x.reshape(b * s, 1024))
    w_full, eye, r16 = make_consts()
    in_maps = []
    for c in range(8):
        xc = np.ascontiguousarray(xf[:, c * 128:(c + 1) * 128])
        tc_ = np.ascontiguousarray(tables[c * 8:(c + 1) * 8].reshape(KLOC * V2, E))
        in_maps.append({"x": xc, "tab": tc_, "w": w_full, "eye": eye, "r16": r16})
    return in_maps


_nc_cache = {}


def kernel(x, tables):
    x = np.asarray(x)
    tables = np.asarray(tables)
    b, s, _ = x.shape
    ntok = b * s
    if ntok not in _nc_cache:
        _nc_cache[ntok] = build_program(ntok=ntok)
    nc = _nc_cache[ntok]
    in_maps = make_in_maps(x, tables)
    res = bass_utils.run_bass_kernel_spmd(nc, in_maps, core_ids=list(range(8)))
    out = np.empty((ntok, 2048), dtype=np.float32)
    for c in range(8):
        out[:, c * 256:(c + 1) * 256] = res.results[c]["out"]
    return out.reshape(b, s, 2048)

